# revision 8
# baseline (speedup 1.0000x reference)
"""Trainium2 Bass kernel for the 3-layer ChebConv (K=3) GCN encoder with
multiplicative noise, sharded over 8 NeuronCores.

Strategy (graph/data parallel, per sharding hint):
- Nodes are permuted by in-degree and dealt round-robin over 160 dest tiles of
  128 rows (20 tiles/core) so every tile carries ~E/160 edges; each core owns
  2560 padded node rows (2500 real).
- prop(z) = segment_sum(norm_w * z[col], row) is computed as a one-hot matmul:
  edges sorted by dest tile, 128-edge chunks; gathered source rows (dma_gather)
  form the moving operand, a built-on-DVE one-hot B matrix (iota == j) is the
  stationary operand, PSUM accumulates the 16 chunks of each dest tile.
- norm_w = -dinv[row]*dinv[col] is separable: gather sources are pre-scaled by
  dinv (z tables), the -dinv[row] factor is applied at PSUM eviction.
- Tx1 / layer outputs are exchanged between cores with AllGather collectives.
- Dense x@W terms run feature-major (weights stationary); prop outputs are
  PE-transposed into feature-major; relu+bias on ACT, noise multiply on DVE.
"""
import numpy as np

N = 20000
E = 320000
NCORES = 8
P = 128
TILES_PER_CORE = 20
ROWS_PAD = TILES_PER_CORE * P          # 2560
NPAD = NCORES * ROWS_PAD               # 20480
NTILES = NCORES * TILES_PER_CORE       # 160
DIMS = [(128, 128), (128, 256), (256, 512)]   # (d_in padded, d_out) per layer
NODE_BLK = 512                          # dense node-block
GROUP_TILES = 4                         # dest tiles per dma_gather


def _build_plan(edges):
    row = edges[0].astype(np.int64)
    col = edges[1].astype(np.int64)
    w = row != col
    deg = np.bincount(row[w], minlength=N).astype(np.float64)
    dinv = np.where(deg > 0, 1.0 / np.sqrt(np.maximum(deg, 1e-12)), 0.0).astype(np.float32)

    order = np.argsort(-deg, kind="stable")
    slot_of = np.empty(N, dtype=np.int64)
    slot_of[order] = (np.arange(N) % NTILES) * P + (np.arange(N) // NTILES)
    orig_of = np.full(NPAD, -1, dtype=np.int64)
    orig_of[slot_of] = np.arange(N)
    dinv_slot = np.zeros(NPAD, dtype=np.float32)
    dinv_slot[slot_of] = dinv

    m = row != col
    er, ec = row[m], col[m]
    sr, sc = slot_of[er], slot_of[ec]
    tile_id = sr // P
    j = (sr % P).astype(np.float32)
    o = np.argsort(tile_id, kind="stable")
    sc, tile_id, j = sc[o], tile_id[o], j[o]
    counts = np.bincount(tile_id, minlength=NTILES)
    K_per_tile = np.maximum(1, np.ceil(counts.reshape(NCORES, TILES_PER_CORE) / P)
                            .astype(np.int64)).max(axis=0)        # [20]
    ktot = int(K_per_tile.sum())
    chunk_off = np.concatenate([[0], np.cumsum(K_per_tile)])
    starts = np.concatenate([[0], np.cumsum(counts)])

    src_idx = np.zeros((NCORES, ktot * P), dtype=np.int16)
    jval = np.full((NCORES, ktot * P), 999.0, dtype=np.float32)
    for c in range(NCORES):
        for t in range(TILES_PER_CORE):
            g = c * TILES_PER_CORE + t
            s, e = starts[g], starts[g + 1]
            o2 = chunk_off[t] * P
            src_idx[c, o2:o2 + (e - s)] = sc[s:e]
            jval[c, o2:o2 + (e - s)] = j[s:e]

    return dict(dinv_slot=dinv_slot, orig_of=orig_of, K_per_tile=K_per_tile,
                ktot=ktot, chunk_off=chunk_off, src_idx=src_idx, jval=jval)


def _build_nc(ktot, chunk_off):
    import concourse.mybir as mybir
    import concourse.tile as tile
    from concourse import bacc
    from concourse.masks import make_identity

    f32 = mybir.dt.float32
    bf16 = mybir.dt.bfloat16
    i16 = mybir.dt.int16
    AF = mybir.ActivationFunctionType
    OP = mybir.AluOpType

    nc = bacc.Bacc(trn_type="TRN2", num_devices=NCORES)

    # ---- I/O -------------------------------------------------------------
    z0f = nc.dram_tensor("z0f", [NPAD, 128], f32, kind="ExternalInput")
    xT0 = nc.dram_tensor("xT0", [128, ROWS_PAD], f32, kind="ExternalInput")
    src16 = nc.dram_tensor("src16", [128, ktot * 8], i16, kind="ExternalInput")
    jarr = nc.dram_tensor("jarr", [128, ktot], f32, kind="ExternalInput")
    sc_tx_d = nc.dram_tensor("sc_tx", [128, TILES_PER_CORE], f32, kind="ExternalInput")
    sc_z1_d = nc.dram_tensor("sc_z1", [128, TILES_PER_CORE], f32, kind="ExternalInput")
    sc_zb_d = nc.dram_tensor("sc_zb", [128, TILES_PER_CORE], f32, kind="ExternalInput")
    wts, biases, noises, outs = [], [], [], []
    for li, (di, do) in enumerate(DIMS):
        wts.append([nc.dram_tensor(f"w{li}{t}", [di, do], f32, kind="ExternalInput")
                    for t in range(3)])
        biases.append(nc.dram_tensor(f"bias{li}", [128, do // 128], f32, kind="ExternalInput"))
        noises.append(nc.dram_tensor(f"noiseT{li}", [do, ROWS_PAD], f32, kind="ExternalInput"))
        outs.append(nc.dram_tensor(f"x{li}T", [do, ROWS_PAD], f32, kind="ExternalOutput"))

    # ---- internal DRAM ---------------------------------------------------
    zdt = [f32, f32, bf16]                 # gather-table dtype per layer
    z1l = [nc.dram_tensor(f"z1l{li}", [ROWS_PAD, di], zdt[li])
           for li, (di, _) in enumerate(DIMS)]
    z1f = [nc.dram_tensor(f"z1f{li}", [NPAD, di], zdt[li], addr_space="Shared")
           for li, (di, _) in enumerate(DIMS)]
    tx1T = [nc.dram_tensor(f"tx1T{li}", [di, ROWS_PAD], f32) for li, (di, _) in enumerate(DIMS)]
    p2T = [nc.dram_tensor(f"p2T{li}", [di, ROWS_PAD], f32) for li, (di, _) in enumerate(DIMS)]
    zxl = [nc.dram_tensor(f"zxl{li}", [ROWS_PAD, do], zdt[li + 1])
           for li, (_, do) in enumerate(DIMS[:2])]
    zxf = [nc.dram_tensor(f"zxf{li}", [NPAD, do], zdt[li + 1], addr_space="Shared")
           for li, (_, do) in enumerate(DIMS[:2])]

    RG = [list(range(NCORES))]

    with tile.TileContext(nc) as tc:
        with (
            tc.tile_pool(name="const", bufs=1) as cp,
            tc.tile_pool(name="g", bufs=2) as gp,
            tc.tile_pool(name="b16", bufs=3) as bp,
            tc.tile_pool(name="ev", bufs=3) as ep,
            tc.tile_pool(name="tt", bufs=3) as tp,
            tc.tile_pool(name="st", bufs=2) as sp,
            tc.tile_pool(name="xo", bufs=3) as xp,
            tc.tile_pool(name="nz", bufs=2) as np_,
            tc.tile_pool(name="ps", bufs=2, space="PSUM") as pp,
            tc.tile_pool(name="pst", bufs=2, space="PSUM") as ppt,
            tc.tile_pool(name="pso", bufs=2, space="PSUM") as ppo,
        ):
            iota_i = cp.tile([128, 16, 128], mybir.dt.int32)
            nc.gpsimd.iota(iota_i[:], pattern=[[0, 16], [1, 128]], base=0, channel_multiplier=0)
            iota_f = cp.tile([128, 16, 128], f32)
            nc.vector.tensor_copy(iota_f[:], iota_i[:])
            iota_b = cp.tile([128, 16, 128], bf16)
            nc.vector.tensor_copy(iota_b[:], iota_i[:])
            ident = cp.tile([128, 128], f32)
            make_identity(nc, ident[:])

            idx_sb = cp.tile([128, ktot * 8], i16)
            nc.sync.dma_start(idx_sb[:], src16[:, :])
            j_sb = cp.tile([128, ktot], f32)
            nc.sync.dma_start(j_sb[:], jarr[:, :])
            j_sb_b = cp.tile([128, ktot], bf16)
            nc.vector.tensor_copy(j_sb_b[:], j_sb[:])
            sc_tx = cp.tile([128, TILES_PER_CORE], f32)
            nc.sync.dma_start(sc_tx[:], sc_tx_d[:, :])
            sc_z1 = cp.tile([128, TILES_PER_CORE], f32)
            nc.sync.dma_start(sc_z1[:], sc_z1_d[:, :])
            sc_zb = cp.tile([128, TILES_PER_CORE], f32)
            nc.sync.dma_start(sc_zb[:], sc_zb_d[:, :])
            # L1/L2 prop outputs stay SBUF-resident (skip the DRAM round-trip)
            tx1T_sb = cp.tile([128, ROWS_PAD], f32, name="tx1T_sb")
            p2T_sb = cp.tile([128, ROWS_PAD], f32, name="p2T_sb")

            def prop(z_src, d, tx_scale, txT_dest, z1_dest=None, z1_scale=None,
                     gdt=f32, txT_sb_dest=None):
                """one propagation pass: txT_dest[d,2560] = scaled segment sum."""
                jj = j_sb if gdt is f32 else j_sb_b
                io = iota_f if gdt is f32 else iota_b
                for t0 in range(0, TILES_PER_CORE, GROUP_TILES):
                    t1 = min(t0 + GROUP_TILES, TILES_PER_CORE)
                    k0, k1 = int(chunk_off[t0]), int(chunk_off[t1])
                    nidx = (k1 - k0) * P
                    g = gp.tile([128, k1 - k0, d], gdt, tag="G", name=f"G_{t0}")
                    nc.gpsimd.dma_gather(
                        out_ap=g[:, :, :], in_ap=z_src[:, :],
                        idxs_ap=idx_sb[:, k0 * 8:k1 * 8],
                        num_idxs=nidx, num_idxs_reg=nidx, elem_size=d,
                        single_packet=False,
                    )
                    for t in range(t0, t1):
                        ck0, ck1 = int(chunk_off[t]), int(chunk_off[t + 1])
                        nch = ck1 - ck0
                        b16 = bp.tile([128, 16, 128], gdt, tag="B", name=f"B_{t}")
                        nc.vector.tensor_tensor(
                            out=b16[:, 0:nch, :],
                            in0=jj[:, ck0:ck1].to_broadcast([128, nch, 128]),
                            in1=io[:, 0:nch, :], op=OP.is_equal)
                        ps = pp.tile([128, d], f32, tag="ps", name=f"ps_{t}")
                        for k in range(nch):
                            nc.tensor.matmul(ps[:], lhsT=b16[:, k, :],
                                             rhs=g[:, ck0 - k0 + k, :],
                                             start=(k == 0), stop=(k == nch - 1))
                        if z1_dest is not None:
                            z1sb = ep.tile([128, d], gdt, tag="z1sb", name=f"z1sb_{t}")
                            nc.scalar.activation(z1sb[:], ps[:], AF.Copy,
                                                 scale=z1_scale[:, t:t + 1])
                            nc.sync.dma_start(z1_dest[t * P:(t + 1) * P, :], z1sb[:])
                        txsb = ep.tile([128, d], f32, tag="txsb", name=f"txsb_{t}")
                        nc.scalar.activation(txsb[:], ps[:], AF.Copy,
                                             scale=tx_scale[:, t:t + 1])
                        for ic in range(d // 128):
                            pst = ppt.tile([128, 128], f32, tag="pst", name=f"pst_{t}_{ic}")
                            nc.tensor.transpose(pst[:], txsb[:, ic * 128:(ic + 1) * 128], ident[:])
                            if txT_sb_dest is not None:
                                nc.scalar.activation(
                                    txT_sb_dest[:, t * P:(t + 1) * P], pst[:], AF.Copy)
                            else:
                                ttsb = tp.tile([128, 128], f32, tag="ttsb", name=f"tt_{t}_{ic}")
                                nc.scalar.activation(ttsb[:], pst[:], AF.Copy)
                                nc.sync.dma_start(
                                    txT_dest[ic * 128:(ic + 1) * 128, t * P:(t + 1) * P],
                                    ttsb[:])

            def dense(li, xT_src, w_sb, d_in, d_out, zx_dest, zx_dt=f32,
                      sb_terms=None):
                nblk = ROWS_PAD // NODE_BLK
                for nb in range(nblk):
                    ns = slice(nb * NODE_BLK, (nb + 1) * NODE_BLK)
                    st = {}
                    for term, src in enumerate([xT_src, tx1T[li], p2T[li]]):
                        if sb_terms is not None and term >= 1:
                            st[(term, 0)] = None  # resident
                            continue
                        for ic in range(d_in // 128):
                            s = sp.tile([128, NODE_BLK], f32, tag=f"st{term}{ic}",
                                        name=f"st{li}_{nb}_{term}_{ic}")
                            nc.sync.dma_start(s[:], src[ic * 128:(ic + 1) * 128, ns])
                            st[(term, ic)] = s
                    for oc in range(d_out // 128):
                        pso = ppo.tile([128, NODE_BLK], f32, tag="pso",
                                       name=f"pso{li}_{nb}_{oc}")
                        pairs = [(term, ic) for term in range(3) for ic in range(d_in // 128)]
                        for i, (term, ic) in enumerate(pairs):
                            if sb_terms is not None and term >= 1:
                                rhs = sb_terms[term - 1][:, ns]
                            else:
                                rhs = st[(term, ic)][:]
                            nc.tensor.matmul(
                                pso[:], lhsT=w_sb[term][ic][:, oc * 128:(oc + 1) * 128],
                                rhs=rhs,
                                start=(i == 0), stop=(i == len(pairs) - 1))
                        xo = xp.tile([128, NODE_BLK], f32, tag="xo", name=f"xo{li}_{nb}_{oc}")
                        nc.scalar.activation(xo[:], pso[:], AF.Relu,
                                             bias=bias_sb[li][:, oc:oc + 1])
                        nz = np_.tile([128, NODE_BLK], f32, tag="nz", name=f"nz{li}_{nb}_{oc}")
                        nc.sync.dma_start(nz[:], noises[li][oc * 128:(oc + 1) * 128, ns])
                        xo2 = xp.tile([128, NODE_BLK], f32, tag="xo2", name=f"xo2{li}_{nb}_{oc}")
                        nc.vector.tensor_tensor(out=xo2[:], in0=xo[:], in1=nz[:], op=OP.mult)
                        nc.sync.dma_start(outs[li][oc * 128:(oc + 1) * 128, ns], xo2[:])
                        if zx_dest is not None:
                            for sub in range(NODE_BLK // 128):
                                t = nb * (NODE_BLK // 128) + sub
                                pst = ppt.tile([128, 128], f32, tag="pst",
                                               name=f"zt{li}_{nb}_{oc}_{sub}")
                                nc.tensor.transpose(pst[:], xo2[:, sub * 128:(sub + 1) * 128],
                                                    ident[:])
                                zsb = tp.tile([128, 128], zx_dt, tag="zsb",
                                              name=f"zsb{li}_{nb}_{oc}_{sub}")
                                nc.scalar.activation(zsb[:], pst[:], AF.Copy,
                                                     scale=sc_zb[:, t:t + 1])
                                nc.sync.dma_start(
                                    zx_dest[t * P:(t + 1) * P, oc * 128:(oc + 1) * 128], zsb[:])

            # weights + biases to SBUF
            w_sb_all, bias_sb = [], []
            for li, (di, do) in enumerate(DIMS):
                terms = []
                for term in range(3):
                    ics = []
                    for ic in range(di // 128):
                        wt = cp.tile([128, do], f32, name=f"w{li}{term}{ic}")
                        nc.sync.dma_start(wt[:], wts[li][term][ic * 128:(ic + 1) * 128, :])
                        ics.append(wt)
                    terms.append(ics)
                w_sb_all.append(terms)
                bt = cp.tile([128, do // 128], f32, name=f"bias_sb{li}")
                nc.sync.dma_start(bt[:], biases[li][:, :])
                bias_sb.append(bt)

            # ---- the 3 layers ------------------------------------------------
            z_in = z0f
            xT_in = xT0
            for li, (di, do) in enumerate(DIMS):
                resident = li < 2    # d_in = 128: keep prop outputs in SBUF
                prop(z_in, di, sc_tx, tx1T[li], z1_dest=z1l[li], z1_scale=sc_z1,
                     gdt=zdt[li], txT_sb_dest=tx1T_sb if resident else None)
                nc.gpsimd.collective_compute(
                    "AllGather", mybir.AluOpType.bypass, replica_groups=RG,
                    ins=[z1l[li][:, :].opt()], outs=[z1f[li][:, :].opt()])
                prop(z1f[li], di, sc_tx, p2T[li], gdt=zdt[li],
                     txT_sb_dest=p2T_sb if resident else None)
                zx_dest = zxl[li] if li < 2 else None
                dense(li, xT_in, w_sb_all[li], di, do, zx_dest,
                      zx_dt=zdt[li + 1] if li < 2 else f32,
                      sb_terms=(tx1T_sb, p2T_sb) if resident else None)
                if li < 2:
                    nc.gpsimd.collective_compute(
                        "AllGather", mybir.AluOpType.bypass, replica_groups=RG,
                        ins=[zxl[li][:, :].opt()], outs=[zxf[li][:, :].opt()])
                    z_in = zxf[li]
                    xT_in = outs[li]

    nc.finalize()
    return nc


def kernel(v, edges, W1, b1, W2, b2, W3, b3, _trace=False):
    import jax
    from concourse.bass_utils import run_bass_kernel_spmd

    v = np.asarray(v, np.float32)
    edges = np.asarray(edges)
    plan = _build_plan(edges)
    dinv_slot = plan["dinv_slot"]
    orig_of = plan["orig_of"]
    valid = orig_of >= 0
    ktot = plan["ktot"]

    # noise (exact same threefry draws as the reference), on CPU
    cpu = jax.devices("cpu")[0]
    with jax.default_device(cpu):
        nk = jax.random.key(42)
        noises = [np.asarray(jax.random.normal(jax.random.fold_in(nk, i + 1), (N, d),
                                               np.float32))
                  for i, d in enumerate([128, 256, 512])]

    Ws = [np.asarray(W1, np.float32), np.asarray(W2, np.float32), np.asarray(W3, np.float32)]
    bs = [np.asarray(b1, np.float32), np.asarray(b2, np.float32), np.asarray(b3, np.float32)]
    W1p = np.zeros((3, 128, 128), np.float32)
    W1p[:, :86] = Ws[0]
    Ws[0] = W1p

    # slot-space padded inputs
    x0 = np.zeros((NPAD, 128), np.float32)
    x0[valid, :86] = v[orig_of[valid]]
    z0 = x0 * dinv_slot[:, None]

    nc = _build_nc(ktot, plan["chunk_off"])

    in_maps = []
    for c in range(NCORES):
        rows = slice(c * ROWS_PAD, (c + 1) * ROWS_PAD)
        src = plan["src_idx"][c]
        m = {
            "z0f": z0,
            "xT0": np.ascontiguousarray(x0[rows].T),
            "src16": np.ascontiguousarray(np.tile(src.reshape(ktot * 8, 16).T, (8, 1))),
            "jarr": np.ascontiguousarray(plan["jval"][c].reshape(ktot, 128).T),
            "sc_tx": np.ascontiguousarray(-dinv_slot[rows].reshape(TILES_PER_CORE, 128).T),
            "sc_z1": np.ascontiguousarray(-(dinv_slot[rows] ** 2).reshape(TILES_PER_CORE, 128).T),
            "sc_zb": np.ascontiguousarray(dinv_slot[rows].reshape(TILES_PER_CORE, 128).T),
        }
        for li, (di, do) in enumerate(DIMS):
            W = Ws[li]
            m[f"w{li}0"] = np.ascontiguousarray(W[0] - W[2])
            m[f"w{li}1"] = np.ascontiguousarray(W[1])
            m[f"w{li}2"] = np.ascontiguousarray(2.0 * W[2])
            m[f"bias{li}"] = np.ascontiguousarray(bs[li].reshape(do // 128, 128).T)
            nz = np.zeros((NPAD, do), np.float32)
            nz[valid] = noises[li][orig_of[valid]]
            m[f"noiseT{li}"] = np.ascontiguousarray(nz[rows].T)
        in_maps.append(m)

    import os
    import time as _time
    _t0 = _time.time()
    res = run_bass_kernel_spmd(nc, in_maps, core_ids=list(range(NCORES)))
    kernel._last_exec_wall = _time.time() - _t0
    if os.environ.get("KBENCH", "0") == "1":
        _t0 = _time.time()
        res = run_bass_kernel_spmd(nc, in_maps, core_ids=list(range(NCORES)))
        kernel._last_exec_wall = _time.time() - _t0
    kernel._last_results = res

    inv = np.argsort(orig_of[valid])  # slot order -> original order
    outs = []
    for li, (_, do) in enumerate(DIMS):
        full = np.concatenate([res.results[c][f"x{li}T"].T for c in range(NCORES)])
        outs.append(np.ascontiguousarray(full[valid][inv][:, :do]))
    return tuple(outs)


# revision 9
# speedup vs baseline: 11.8683x; 11.8683x over previous
"""Trainium2 Bass kernel for the 3-layer ChebConv (K=3) GCN encoder with
multiplicative noise, sharded over 8 NeuronCores.

Strategy (graph/data parallel, per sharding hint):
- Nodes are permuted by in-degree and dealt round-robin over 160 dest tiles of
  128 rows (20 tiles/core) so every tile carries ~E/160 edges; each core owns
  2560 padded node rows (2500 real).
- prop(z) = segment_sum(norm_w * z[col], row) is computed as a one-hot matmul:
  edges sorted by dest tile, 128-edge chunks; gathered source rows (dma_gather)
  form the moving operand, a built-on-DVE one-hot B matrix (iota == j) is the
  stationary operand, PSUM accumulates the 16 chunks of each dest tile.
- norm_w = -dinv[row]*dinv[col] is separable: gather sources are pre-scaled by
  dinv (z tables), the -dinv[row] factor is applied at PSUM eviction.
- Tx1 / layer outputs are exchanged between cores with AllGather collectives.
- Dense x@W terms run feature-major (weights stationary); prop outputs are
  PE-transposed into feature-major; relu+bias on ACT, noise multiply on DVE.
"""
import numpy as np

N = 20000
E = 320000
NCORES = 8
P = 128
TILES_PER_CORE = 20
ROWS_PAD = TILES_PER_CORE * P          # 2560
NPAD = NCORES * ROWS_PAD               # 20480
NTILES = NCORES * TILES_PER_CORE       # 160
DIMS = [(128, 128), (128, 256), (256, 512)]   # (d_in padded, d_out) per layer
NODE_BLK = 512                          # dense node-block
GROUP_TILES = 4                         # dest tiles per dma_gather


def _build_plan(edges):
    row = edges[0].astype(np.int64)
    col = edges[1].astype(np.int64)
    w = row != col
    deg = np.bincount(row[w], minlength=N).astype(np.float64)
    dinv = np.where(deg > 0, 1.0 / np.sqrt(np.maximum(deg, 1e-12)), 0.0).astype(np.float32)

    order = np.argsort(-deg, kind="stable")
    slot_of = np.empty(N, dtype=np.int64)
    slot_of[order] = (np.arange(N) % NTILES) * P + (np.arange(N) // NTILES)
    orig_of = np.full(NPAD, -1, dtype=np.int64)
    orig_of[slot_of] = np.arange(N)
    dinv_slot = np.zeros(NPAD, dtype=np.float32)
    dinv_slot[slot_of] = dinv

    m = row != col
    er, ec = row[m], col[m]
    sr, sc = slot_of[er], slot_of[ec]
    tile_id = sr // P
    j = (sr % P).astype(np.float32)
    o = np.argsort(tile_id, kind="stable")
    sc, tile_id, j = sc[o], tile_id[o], j[o]
    counts = np.bincount(tile_id, minlength=NTILES)
    K_per_tile = np.maximum(1, np.ceil(counts.reshape(NCORES, TILES_PER_CORE) / P)
                            .astype(np.int64)).max(axis=0)        # [20]
    ktot = int(K_per_tile.sum())
    chunk_off = np.concatenate([[0], np.cumsum(K_per_tile)])
    starts = np.concatenate([[0], np.cumsum(counts)])

    src_idx = np.zeros((NCORES, ktot * P), dtype=np.int16)
    jval = np.full((NCORES, ktot * P), 999.0, dtype=np.float32)
    for c in range(NCORES):
        for t in range(TILES_PER_CORE):
            g = c * TILES_PER_CORE + t
            s, e = starts[g], starts[g + 1]
            o2 = chunk_off[t] * P
            src_idx[c, o2:o2 + (e - s)] = sc[s:e]
            jval[c, o2:o2 + (e - s)] = j[s:e]

    return dict(dinv_slot=dinv_slot, orig_of=orig_of, K_per_tile=K_per_tile,
                ktot=ktot, chunk_off=chunk_off, src_idx=src_idx, jval=jval)


def _build_nc(ktot, chunk_off):
    import concourse.mybir as mybir
    import concourse.tile as tile
    from concourse import bacc
    from concourse.masks import make_identity

    f32 = mybir.dt.float32
    bf16 = mybir.dt.bfloat16
    i16 = mybir.dt.int16
    AF = mybir.ActivationFunctionType
    OP = mybir.AluOpType

    nc = bacc.Bacc(trn_type="TRN2", num_devices=NCORES)

    # ---- I/O -------------------------------------------------------------
    z0f = nc.dram_tensor("z0f", [NPAD, 128], f32, kind="ExternalInput")
    xT0 = nc.dram_tensor("xT0", [128, ROWS_PAD], f32, kind="ExternalInput")
    src16 = nc.dram_tensor("src16", [128, ktot * 8], i16, kind="ExternalInput")
    jarr = nc.dram_tensor("jarr", [128, ktot], f32, kind="ExternalInput")
    sc_tx_d = nc.dram_tensor("sc_tx", [128, TILES_PER_CORE], f32, kind="ExternalInput")
    sc_z1_d = nc.dram_tensor("sc_z1", [128, TILES_PER_CORE], f32, kind="ExternalInput")
    sc_zb_d = nc.dram_tensor("sc_zb", [128, TILES_PER_CORE], f32, kind="ExternalInput")
    wts, biases, noises, outs = [], [], [], []
    for li, (di, do) in enumerate(DIMS):
        wts.append([nc.dram_tensor(f"w{li}{t}", [di, do], f32, kind="ExternalInput")
                    for t in range(3)])
        biases.append(nc.dram_tensor(f"bias{li}", [128, do // 128], f32, kind="ExternalInput"))
        noises.append(nc.dram_tensor(f"noiseT{li}", [do, ROWS_PAD], f32, kind="ExternalInput"))
        outs.append(nc.dram_tensor(f"x{li}T", [do, ROWS_PAD], f32, kind="ExternalOutput"))

    # ---- internal DRAM ---------------------------------------------------
    zdt = [f32, f32, bf16]                 # gather-table dtype per layer
    z1l = [nc.dram_tensor(f"z1l{li}", [ROWS_PAD, di], zdt[li])
           for li, (di, _) in enumerate(DIMS)]
    z1f = [nc.dram_tensor(f"z1f{li}", [NPAD, di], zdt[li], addr_space="Shared")
           for li, (di, _) in enumerate(DIMS)]
    tx1T = [nc.dram_tensor(f"tx1T{li}", [di, ROWS_PAD], f32) for li, (di, _) in enumerate(DIMS)]
    p2T = [nc.dram_tensor(f"p2T{li}", [di, ROWS_PAD], f32) for li, (di, _) in enumerate(DIMS)]
    zxl = [nc.dram_tensor(f"zxl{li}", [ROWS_PAD, do], zdt[li + 1])
           for li, (_, do) in enumerate(DIMS[:2])]
    zxf = [nc.dram_tensor(f"zxf{li}", [NPAD, do], zdt[li + 1], addr_space="Shared")
           for li, (_, do) in enumerate(DIMS[:2])]

    RG = [list(range(NCORES))]

    with tile.TileContext(nc) as tc:
        with (
            tc.tile_pool(name="const", bufs=1) as cp,
            tc.tile_pool(name="g", bufs=2) as gp,
            tc.tile_pool(name="b16", bufs=3) as bp,
            tc.tile_pool(name="ev", bufs=3) as ep,
            tc.tile_pool(name="tt", bufs=3) as tp,
            tc.tile_pool(name="st", bufs=2) as sp,
            tc.tile_pool(name="xo", bufs=3) as xp,
            tc.tile_pool(name="nz", bufs=2) as np_,
            tc.tile_pool(name="ps", bufs=3, space="PSUM") as pp,
            tc.tile_pool(name="pst", bufs=2, space="PSUM") as ppt,
            tc.tile_pool(name="pso", bufs=3, space="PSUM") as ppo,
        ):
            iota_i = cp.tile([128, 16, 128], mybir.dt.int32)
            nc.gpsimd.iota(iota_i[:], pattern=[[0, 16], [1, 128]], base=0, channel_multiplier=0)
            iota_f = cp.tile([128, 16, 128], f32)
            nc.vector.tensor_copy(iota_f[:], iota_i[:])
            iota_b = cp.tile([128, 16, 128], bf16)
            nc.vector.tensor_copy(iota_b[:], iota_i[:])
            ident = cp.tile([128, 128], f32)
            make_identity(nc, ident[:])

            idx_sb = cp.tile([128, ktot * 8], i16)
            nc.sync.dma_start(idx_sb[:], src16[:, :])
            j_sb = cp.tile([128, ktot], f32)
            nc.sync.dma_start(j_sb[:], jarr[:, :])
            j_sb_b = cp.tile([128, ktot], bf16)
            nc.vector.tensor_copy(j_sb_b[:], j_sb[:])
            sc_tx = cp.tile([128, TILES_PER_CORE], f32)
            nc.sync.dma_start(sc_tx[:], sc_tx_d[:, :])
            sc_z1 = cp.tile([128, TILES_PER_CORE], f32)
            nc.sync.dma_start(sc_z1[:], sc_z1_d[:, :])
            sc_zb = cp.tile([128, TILES_PER_CORE], f32)
            nc.sync.dma_start(sc_zb[:], sc_zb_d[:, :])
            # L1/L2 prop outputs stay SBUF-resident (skip the DRAM round-trip)
            tx1T_sb = cp.tile([128, ROWS_PAD], f32, name="tx1T_sb")
            p2T_sb = cp.tile([128, ROWS_PAD], f32, name="p2T_sb")

            def prop(z_src, d, tx_scale, txT_dest, z1_dest=None, z1_scale=None,
                     gdt=f32, txT_sb_dest=None, group_cb=None):
                """one propagation pass: txT_dest[d,2560] = scaled segment sum."""
                jj = j_sb if gdt is f32 else j_sb_b
                io = iota_f if gdt is f32 else iota_b
                for gi, t0 in enumerate(range(0, TILES_PER_CORE, GROUP_TILES)):
                    t1 = min(t0 + GROUP_TILES, TILES_PER_CORE)
                    k0, k1 = int(chunk_off[t0]), int(chunk_off[t1])
                    nidx = (k1 - k0) * P
                    g = gp.tile([128, k1 - k0, d], gdt, tag="G", name=f"G_{t0}")
                    nc.gpsimd.dma_gather(
                        out_ap=g[:, :, :], in_ap=z_src[:, :],
                        idxs_ap=idx_sb[:, k0 * 8:k1 * 8],
                        num_idxs=nidx, num_idxs_reg=nidx, elem_size=d,
                        single_packet=False,
                    )
                    for t in range(t0, t1):
                        ck0, ck1 = int(chunk_off[t]), int(chunk_off[t + 1])
                        nch = ck1 - ck0
                        b16 = bp.tile([128, 16, 128], gdt, tag="B", name=f"B_{t}")
                        nc.vector.tensor_tensor(
                            out=b16[:, 0:nch, :],
                            in0=jj[:, ck0:ck1].to_broadcast([128, nch, 128]),
                            in1=io[:, 0:nch, :], op=OP.is_equal)
                        ps = pp.tile([128, d], f32, tag="ps", name=f"ps_{t}")
                        for k in range(nch):
                            nc.tensor.matmul(ps[:], lhsT=b16[:, k, :],
                                             rhs=g[:, ck0 - k0 + k, :],
                                             start=(k == 0), stop=(k == nch - 1))
                        if z1_dest is not None:
                            z1sb = ep.tile([128, d], gdt, tag="z1sb", name=f"z1sb_{t}")
                            nc.scalar.activation(z1sb[:], ps[:], AF.Copy,
                                                 scale=z1_scale[:, t:t + 1])
                            nc.sync.dma_start(z1_dest[t * P:(t + 1) * P, :], z1sb[:])
                        txsb = ep.tile([128, d], f32, tag="txsb", name=f"txsb_{t}")
                        nc.scalar.activation(txsb[:], ps[:], AF.Copy,
                                             scale=tx_scale[:, t:t + 1])
                        for ic in range(d // 128):
                            pst = ppt.tile([128, 128], f32, tag="pst", name=f"pst_{t}_{ic}")
                            nc.tensor.transpose(pst[:], txsb[:, ic * 128:(ic + 1) * 128], ident[:])
                            if txT_sb_dest is not None:
                                nc.scalar.activation(
                                    txT_sb_dest[:, t * P:(t + 1) * P], pst[:], AF.Copy)
                            else:
                                ttsb = tp.tile([128, 128], f32, tag="ttsb", name=f"tt_{t}_{ic}")
                                nc.scalar.activation(ttsb[:], pst[:], AF.Copy)
                                nc.sync.dma_start(
                                    txT_dest[ic * 128:(ic + 1) * 128, t * P:(t + 1) * P],
                                    ttsb[:])
                    if group_cb is not None:
                        group_cb(gi)

            def dense_block(li, nb, xT_src, w_sb, d_in, d_out, zx_dest, zx_dt=f32,
                            sb_terms=None):
                    ns = slice(nb * NODE_BLK, (nb + 1) * NODE_BLK)
                    st = {}
                    for term, src in enumerate([xT_src, tx1T[li], p2T[li]]):
                        if sb_terms is not None and term >= 1:
                            st[(term, 0)] = None  # resident
                            continue
                        for ic in range(d_in // 128):
                            s = sp.tile([128, NODE_BLK], f32, tag=f"st{term}{ic}",
                                        name=f"st{li}_{nb}_{term}_{ic}")
                            nc.sync.dma_start(s[:], src[ic * 128:(ic + 1) * 128, ns])
                            st[(term, ic)] = s
                    for oc in range(d_out // 128):
                        pso = ppo.tile([128, NODE_BLK], f32, tag="pso",
                                       name=f"pso{li}_{nb}_{oc}")
                        pairs = [(term, ic) for term in range(3) for ic in range(d_in // 128)]
                        for i, (term, ic) in enumerate(pairs):
                            if sb_terms is not None and term >= 1:
                                rhs = sb_terms[term - 1][:, ns]
                            else:
                                rhs = st[(term, ic)][:]
                            nc.tensor.matmul(
                                pso[:], lhsT=w_sb[term][ic][:, oc * 128:(oc + 1) * 128],
                                rhs=rhs,
                                start=(i == 0), stop=(i == len(pairs) - 1))
                        xo = xp.tile([128, NODE_BLK], f32, tag="xo", name=f"xo{li}_{nb}_{oc}")
                        nc.scalar.activation(xo[:], pso[:], AF.Relu,
                                             bias=bias_sb[li][:, oc:oc + 1])
                        nz = np_.tile([128, NODE_BLK], f32, tag="nz", name=f"nz{li}_{nb}_{oc}")
                        nc.sync.dma_start(nz[:], noises[li][oc * 128:(oc + 1) * 128, ns])
                        xo2 = xp.tile([128, NODE_BLK], f32, tag="xo2", name=f"xo2{li}_{nb}_{oc}")
                        nc.vector.tensor_tensor(out=xo2[:], in0=xo[:], in1=nz[:], op=OP.mult)
                        nc.sync.dma_start(outs[li][oc * 128:(oc + 1) * 128, ns], xo2[:])
                        if zx_dest is not None:
                            for sub in range(NODE_BLK // 128):
                                t = nb * (NODE_BLK // 128) + sub
                                pst = ppt.tile([128, 128], f32, tag="pst",
                                               name=f"zt{li}_{nb}_{oc}_{sub}")
                                nc.tensor.transpose(pst[:], xo2[:, sub * 128:(sub + 1) * 128],
                                                    ident[:])
                                zsb = tp.tile([128, 128], zx_dt, tag="zsb",
                                              name=f"zsb{li}_{nb}_{oc}_{sub}")
                                nc.scalar.activation(zsb[:], pst[:], AF.Copy,
                                                     scale=sc_zb[:, t:t + 1])
                                nc.sync.dma_start(
                                    zx_dest[t * P:(t + 1) * P, oc * 128:(oc + 1) * 128], zsb[:])

            # weights + biases to SBUF
            w_sb_all, bias_sb = [], []
            for li, (di, do) in enumerate(DIMS):
                terms = []
                for term in range(3):
                    ics = []
                    for ic in range(di // 128):
                        wt = cp.tile([128, do], f32, name=f"w{li}{term}{ic}")
                        nc.sync.dma_start(wt[:], wts[li][term][ic * 128:(ic + 1) * 128, :])
                        ics.append(wt)
                    terms.append(ics)
                w_sb_all.append(terms)
                bt = cp.tile([128, do // 128], f32, name=f"bias_sb{li}")
                nc.sync.dma_start(bt[:], biases[li][:, :])
                bias_sb.append(bt)

            # ---- the 3 layers ------------------------------------------------
            z_in = z0f
            xT_in = xT0
            for li, (di, do) in enumerate(DIMS):
                resident = li < 2    # d_in = 128: keep prop outputs in SBUF
                prop(z_in, di, sc_tx, tx1T[li], z1_dest=z1l[li], z1_scale=sc_z1,
                     gdt=zdt[li], txT_sb_dest=tx1T_sb if resident else None)
                nc.gpsimd.collective_compute(
                    "AllGather", mybir.AluOpType.bypass, replica_groups=RG,
                    ins=[z1l[li][:, :].opt()], outs=[z1f[li][:, :].opt()])
                zx_dest = zxl[li] if li < 2 else None

                def _cb(nb, li=li, di=di, do=do, zx_dest=zx_dest, res=resident):
                    dense_block(li, nb, xT_in, w_sb_all[li], di, do, zx_dest,
                                zx_dt=zdt[li + 1] if li < 2 else f32,
                                sb_terms=(tx1T_sb, p2T_sb) if res else None)

                prop(z1f[li], di, sc_tx, p2T[li], gdt=zdt[li],
                     txT_sb_dest=p2T_sb if resident else None, group_cb=_cb)
                if li < 2:
                    nc.gpsimd.collective_compute(
                        "AllGather", mybir.AluOpType.bypass, replica_groups=RG,
                        ins=[zxl[li][:, :].opt()], outs=[zxf[li][:, :].opt()])
                    z_in = zxf[li]
                    xT_in = outs[li]

    nc.finalize()
    return nc


def kernel(v, edges, W1, b1, W2, b2, W3, b3, _trace=False):
    import jax
    from concourse.bass_utils import run_bass_kernel_spmd

    v = np.asarray(v, np.float32)
    edges = np.asarray(edges)
    plan = _build_plan(edges)
    dinv_slot = plan["dinv_slot"]
    orig_of = plan["orig_of"]
    valid = orig_of >= 0
    ktot = plan["ktot"]

    # noise (exact same threefry draws as the reference), on CPU
    cpu = jax.devices("cpu")[0]
    with jax.default_device(cpu):
        nk = jax.random.key(42)
        noises = [np.asarray(jax.random.normal(jax.random.fold_in(nk, i + 1), (N, d),
                                               np.float32))
                  for i, d in enumerate([128, 256, 512])]

    Ws = [np.asarray(W1, np.float32), np.asarray(W2, np.float32), np.asarray(W3, np.float32)]
    bs = [np.asarray(b1, np.float32), np.asarray(b2, np.float32), np.asarray(b3, np.float32)]
    W1p = np.zeros((3, 128, 128), np.float32)
    W1p[:, :86] = Ws[0]
    Ws[0] = W1p

    # slot-space padded inputs
    x0 = np.zeros((NPAD, 128), np.float32)
    x0[valid, :86] = v[orig_of[valid]]
    z0 = x0 * dinv_slot[:, None]

    nc = _build_nc(ktot, plan["chunk_off"])

    in_maps = []
    for c in range(NCORES):
        rows = slice(c * ROWS_PAD, (c + 1) * ROWS_PAD)
        src = plan["src_idx"][c]
        m = {
            "z0f": z0,
            "xT0": np.ascontiguousarray(x0[rows].T),
            "src16": np.ascontiguousarray(np.tile(src.reshape(ktot * 8, 16).T, (8, 1))),
            "jarr": np.ascontiguousarray(plan["jval"][c].reshape(ktot, 128).T),
            "sc_tx": np.ascontiguousarray(-dinv_slot[rows].reshape(TILES_PER_CORE, 128).T),
            "sc_z1": np.ascontiguousarray(-(dinv_slot[rows] ** 2).reshape(TILES_PER_CORE, 128).T),
            "sc_zb": np.ascontiguousarray(dinv_slot[rows].reshape(TILES_PER_CORE, 128).T),
        }
        for li, (di, do) in enumerate(DIMS):
            W = Ws[li]
            m[f"w{li}0"] = np.ascontiguousarray(W[0] - W[2])
            m[f"w{li}1"] = np.ascontiguousarray(W[1])
            m[f"w{li}2"] = np.ascontiguousarray(2.0 * W[2])
            m[f"bias{li}"] = np.ascontiguousarray(bs[li].reshape(do // 128, 128).T)
            nz = np.zeros((NPAD, do), np.float32)
            nz[valid] = noises[li][orig_of[valid]]
            m[f"noiseT{li}"] = np.ascontiguousarray(nz[rows].T)
        in_maps.append(m)

    import os
    import time as _time
    _t0 = _time.time()
    res = run_bass_kernel_spmd(nc, in_maps, core_ids=list(range(NCORES)))
    kernel._last_exec_wall = _time.time() - _t0
    if os.environ.get("KBENCH", "0") == "1":
        _t0 = _time.time()
        res = run_bass_kernel_spmd(nc, in_maps, core_ids=list(range(NCORES)))
        kernel._last_exec_wall = _time.time() - _t0
    kernel._last_results = res

    inv = np.argsort(orig_of[valid])  # slot order -> original order
    outs = []
    for li, (_, do) in enumerate(DIMS):
        full = np.concatenate([res.results[c][f"x{li}T"].T for c in range(NCORES)])
        outs.append(np.ascontiguousarray(full[valid][inv][:, :do]))
    return tuple(outs)
